# revision 1
# baseline (speedup 1.0000x reference)
"""Trainium2 Bass kernel for the GNN message-passing model.

Math (reference):
    base[b,s,t,j] = x[b,s,t,j]            (j<4)
    extra[b,s,t,c] = x[b,s,t,4+c]
    h_pre[b,c,s,h] = sum_{t,j} base[b,s,t,j]*mW1[5t+j,h]
                   + sum_t extra[b,s,t,c]*mW1[5t+4,h] + mb1[h]
    msg_sum[b,c,:] = sum_s relu(h_pre[b,c,s,:]) @ mW2 + N*mb2
    out = MLP(concat(msg_sum, x[:,:,-1,:4]))

Decomposition used here:
  * A[b,s,h] = base-part + mb1 is shared across all columns c -> precomputed
    on host (21 MFLOP of ~26 GFLOP total) and folded into the matmul as an
    extra contraction row against a ones-row in the rhs.
  * Per (b,s) pair the device does ONE matmul [K=11, M=128h, N=512c]
    producing h_pre for all columns, then relu, then accumulates over s.
  * sum_s(relu(h) @ mW2) == (sum_s relu(h)) @ mW2, and further the mW2
    matmul itself can BE the s-accumulator: PSUM accumulation of
    mW2.T @ relu_tile over s (bf16 relu tiles; the tiny mW2 in bf16).
    Alternating pairs instead use a fused DVE scalar_tensor_tensor
    (H = max(psum,0) + H, fp32) to balance ACT/DVE/PE load.
  * The big per-pair matmul runs in bf16 with a hi/lo split on the
    contraction axis (K=33: Whi*xhi + Whi*xlo + Wlo*xhi): PE streams the
    512 columns at 1 cycle/row regardless of K, so this gets bf16 speed
    (fp32 is 4 cycles/row) at ~1e-5 product error.
  * Sharding: data-parallel over the source axis s (512 -> 64 per core);
    each core produces a partial msg[b,32,c] for all columns; host sums the
    8 partials and runs the tiny update MLP (0.15% of FLOPs) in numpy.
"""

import os
import numpy as np

import concourse.bass as bass
import concourse.mybir as mybir
from concourse.tile import TileContext
from concourse.bass_utils import run_bass_kernel_spmd

B, N, T, F = 4, 512, 10, 516
HID, MSG = 128, 32
NCORES = 8
SLOC = N // NCORES          # source rows per core
K1 = T + 1                  # 10 extra-feature rows + 1 ones-row (bias fold)
KP = 3 * K1                 # bf16 hi/lo split: [Whi*xhi, Whi*xlo, Wlo*xhi]
F32 = mybir.dt.float32
F32R = mybir.dt.float32r
BF16 = mybir.dt.bfloat16

# fraction of (b,s) pairs whose relu runs on the scalar engine (ACT) with the
# accumulate done on the tensor engine; the rest use the fused DVE op.
# pair-type pattern: "A" = ACT relu + PE (mW2) accumulate,
# "B" = fused DVE accumulate (hacc = max(psum,0) + hacc).
PATTERN = ["A", "B"] * 8
MM1_F32R = True     # use float32r for the big per-pair matmul
ACC_BF16 = True     # bf16 relu output + bf16 mW2 accumulate matmul

_prog = None
last_results = None

# Tile emits semaphore waits for same-engine WAW/RAW deps (e.g. an ACT op
# waiting on the ACT sem for a pool buffer recycled from an older ACT write).
# Compute engines execute strictly in order, so these waits are redundant --
# and they overflow the 1-slot sync-wait budget of several ISA structs
# (ACTIVATE, TensorScalarPtr). Strip them post-scheduling.
_STRIP_TYPES = {
    "InstActivation", "InstTensorScalarPtr", "InstTensorTensor",
    "InstTensorCopy", "InstTensorReduce", "InstMatmult", "InstMemSet",
}
_ENG2SEM = None


def _strip_self_waits(nc):
    global _ENG2SEM
    if _ENG2SEM is None:
        _ENG2SEM = {
            mybir.EngineType.PE: "PE_",
            mybir.EngineType.Activation: "Activation_",
            mybir.EngineType.DVE: "DVE_",
            mybir.EngineType.Pool: "Pool_",
        }
    for fn in nc.m.functions:
        for blk in fn.blocks:
            for inst in blk.instructions:
                if type(inst).__name__ not in _STRIP_TYPES:
                    continue
                si = inst.sync_info
                if si is None or not si.on_wait:
                    continue
                pre = _ENG2SEM.get(inst.engine)
                if pre is None:
                    continue
                kept = [w for w in si.on_wait if not (w.ant_name or "").startswith(pre)]
                if len(kept) != len(si.on_wait):
                    si.on_wait = kept
    # Chunk-load DMAs: the WAR wait on the engine that read the recycled
    # buffer transitively dominates the WAW wait on the DMA that previously
    # filled it (that engine's reads each waited on that DMA themselves).
    eng_sems = ("PE_", "Activation_", "DVE_", "Pool_")
    for fn in nc.m.functions:
        for blk in fn.blocks:
            for inst in blk.instructions:
                if type(inst).__name__ != "InstDMACopy":
                    continue
                si = inst.sync_info
                if si is None or not si.on_wait:
                    continue
                has_eng = any((w.ant_name or "").startswith(eng_sems) for w in si.on_wait)
                if not has_eng:
                    continue
                kept = [
                    w for w in si.on_wait
                    if not (w.ant_name or "").startswith(("DMAHW", "DMASW"))
                ]
                if len(kept) != len(si.on_wait):
                    si.on_wait = kept
    # Kernel-tail Drain: waits on every DMA queue overflow the CTRL struct's
    # wait budget. Input-DMA waits are dominated by the engine waits (each
    # load was read by a compute engine before the drain); only the queues
    # carrying the output DMAs must be waited on directly.
    out_sems = set()
    for fn in nc.m.functions:
        for blk in fn.blocks:
            for inst in blk.instructions:
                if type(inst).__name__ != "InstDMACopy":
                    continue
                outs = getattr(inst, "outs", None) or []
                to_dram = any("msg_out" in (getattr(o, "memref", "") or "")
                              for o in outs)
                si = inst.sync_info
                if to_dram and si and si.on_update:
                    for u in si.on_update:
                        out_sems.add(u.ant_name)
    drain_split = 0
    for fn in nc.m.functions:
        for blk in fn.blocks:
            for ii in range(len(blk.instructions)):
                inst = blk.instructions[ii]
                if type(inst).__name__ != "InstDrain":
                    continue
                si = inst.sync_info
                if si is None or not si.on_wait or len(si.on_wait) <= 1:
                    continue
                waits = [
                    w for w in si.on_wait
                    if not (w.ant_name or "").startswith(("DMAHW", "DMASW"))
                    or w.ant_name in out_sems
                ]
                # split into a chain of drains with one wait each (the SP
                # CTRL struct has a single sync-wait slot)
                pre = []
                while len(waits) > 1:
                    chunk, waits = waits[:1], waits[1:]
                    d = mybir.InstDrain(
                        name=f"{inst.name}_split{drain_split}", ins=[], outs=[],
                        sync_info=mybir.SyncInfo(on_wait=chunk, on_update=[]),
                    )
                    d.engine = inst.engine
                    drain_split += 1
                    pre.append(d)
                si.on_wait = waits
                for d in reversed(pre):
                    blk.instructions.insert(ii, d)
                break


def _build_program():
    nc = bass.Bass(trn_type="TRN2")
    # packed input: per (b, s) an [K1, N+HID] block -- first N cols are the
    # matmul rhs (extra features + ones row), last HID cols the per-pair lhsT
    # (W1x rows + folded bias row). One tensor -> one DMA sem per chunk.
    # packed bf16 tensor: contraction rows tripled for the hi/lo split --
    # the matmul streams N=512 columns regardless of K, so K=33 bf16 runs at
    # 1 cycle/row (240ns) with ~1e-5 product error (vs f32r's 324ns / ~4e-4)
    ext = nc.dram_tensor("ext", [B, SLOC, KP, N + HID], BF16, kind="ExternalInput")
    w2 = nc.dram_tensor("w2", [HID, MSG], F32, kind="ExternalInput")
    w2b = nc.dram_tensor("w2b", [HID, MSG], BF16, kind="ExternalInput")
    msg_out = nc.dram_tensor("msg_out", [B, MSG, N], F32, kind="ExternalOutput")

    CH = 16  # source rows per DMA chunk
    with TileContext(nc) as tc:
        with (
            tc.tile_pool(name="const", bufs=1) as constp,
            tc.tile_pool(name="big", bufs=2) as bigp,
            tc.tile_pool(name="relua", bufs=4) as rap,   # ACT-relu'd, read by PE
            tc.tile_pool(name="hacc", bufs=2) as hp,
            tc.tile_pool(name="out", bufs=4) as outp,
            tc.tile_pool(name="ps", bufs=6, space="PSUM") as pp,
            tc.tile_pool(name="pwarm", bufs=1, space="PSUM") as pwp,
            tc.tile_pool(name="pacc", bufs=1, space="PSUM") as pa,
        ):
            w2t = constp.tile([HID, MSG], F32)
            nc.sync.dma_start(w2t[:], w2[:])
            w2bt = constp.tile([HID, MSG], BF16, tag="w2bt")
            nc.sync.dma_start(w2bt[:], w2b[:])
            # warmup touch of w2t on PE so later macc matmuls don't need a
            # DMA wait on top of their relu-tile wait
            warm = pwp.tile([MSG, 1], F32, tag="warm")
            nc.tensor.matmul(warm[:], w2t[:], w2t[:, :1], start=True, stop=True)
            warm2 = pwp.tile([MSG, 1], F32, tag="warm")
            nc.tensor.matmul(warm2[:], w2bt[:], w2bt[:, :1], start=True, stop=True)

            for b in range(B):
                hacc = None
                macc = pa.tile([MSG, N], F32, tag="macc")
                nmm = 0
                hacc_init = False
                for g in range(SLOC // CH):
                    big_t = bigp.tile([KP, CH, N + HID], BF16, tag="big")
                    nc.sync.dma_start(
                        big_t[:],
                        ext[b, g * CH:(g + 1) * CH].rearrange("s k c -> k s c"),
                    )
                    # tiny PE touch of the fresh chunk: absorbs the DMA wait
                    # so the first real matmul only waits on the PSUM recycle
                    wt = pwp.tile([MSG, 1], F32, tag="warm")
                    nc.tensor.matmul(
                        wt[:, :1], big_t[:, 0, :MSG], big_t[:, 0, :1],
                        start=True, stop=True,
                    )
                    for si in range(CH):
                        s = g * CH + si
                        p = b * SLOC + s
                        ty = PATTERN[p % len(PATTERN)]
                        ps = pp.tile([HID, N], F32, tag="ps")
                        nc.tensor.matmul(
                            ps[:], big_t[:, si, N:N + HID], big_t[:, si, :N],
                            start=True, stop=True,
                        )
                        if ty == "A":
                            r = rap.tile([HID, N], BF16 if ACC_BF16 else F32, tag="relua")
                            nc.scalar.activation(
                                r[:], ps[:], mybir.ActivationFunctionType.Relu
                            )
                            nc.tensor.matmul(
                                macc[:], w2bt[:] if ACC_BF16 else w2t[:], r[:],
                                start=(nmm == 0), stop=False,
                                skip_group_check=True,
                            )
                            nmm += 1
                        else:  # "B": fused DVE relu+accumulate from PSUM
                            if not hacc_init:
                                hacc = hp.tile([HID, N], F32, tag="hacc")
                                nc.vector.tensor_scalar(
                                    hacc[:], ps[:], 0.0, None,
                                    op0=mybir.AluOpType.max,
                                )
                                hacc_init = True
                            else:
                                nc.vector.scalar_tensor_tensor(
                                    hacc[:], ps[:], 0.0, hacc[:],
                                    op0=mybir.AluOpType.max,
                                    op1=mybir.AluOpType.add,
                                )
                # fold the DVE-accumulated part through mW2 as well
                if hacc_init:
                    nc.tensor.matmul(
                        macc[:], w2t[:], hacc[:],
                        start=(nmm == 0), stop=True,
                        skip_group_check=True,
                    )
                ot = outp.tile([MSG, N], F32, tag="ot")
                nc.scalar.copy(ot[:], macc[:])
                nc.sync.dma_start(msg_out[b], ot[:])
    _strip_self_waits(nc)
    return nc


def _get_prog():
    global _prog
    if _prog is None:
        _prog = _build_program()
    return _prog


def kernel(x, mW1, mb1, mW2, mb2, iW1, ib1, iW2, ib2):
    global last_results
    x = np.ascontiguousarray(np.asarray(x, dtype=np.float32))
    mW1 = np.asarray(mW1, dtype=np.float32)
    mb1 = np.asarray(mb1, dtype=np.float32)
    mW2 = np.ascontiguousarray(np.asarray(mW2, dtype=np.float32))
    mb2 = np.asarray(mb2, dtype=np.float32)

    # host prep: A[b,s,h] = base_flat @ W1b + mb1 (tiny), weight slices
    base = x[:, :, :, :4]                                  # [B,N,T,4]
    base_flat = base.reshape(B, N, T * 4)
    W1b = mW1.reshape(T, 5, HID)[:, :4, :].reshape(T * 4, HID)
    W1x = np.ascontiguousarray(mW1.reshape(T, 5, HID)[:, 4, :])   # [T,HID]
    A = base_flat @ W1b + mb1                              # [B,N,HID]
    import ml_dtypes
    mW2b = mW2.astype(ml_dtypes.bfloat16)

    # per-core inputs: bf16 hi/lo split on the contraction axis.
    # rows [0:11]  = (Whi, xhi), rows [11:22] = (Whi, xlo),
    # rows [22:33] = (Wlo, xhi)  ->  Whi*xhi + Whi*xlo + Wlo*xhi ~ W*x
    bf16 = ml_dtypes.bfloat16
    in_maps = []
    for k in range(NCORES):
        sl = slice(k * SLOC, (k + 1) * SLOC)
        rhs = np.empty((B, SLOC, K1, N), dtype=np.float32)
        rhs[:, :, :T, :] = x[:, sl, :, 4:4 + N]
        rhs[:, :, T, :] = 1.0
        lhs = np.empty((B, SLOC, K1, HID), dtype=np.float32)
        lhs[:, :, :T, :] = W1x[None, None, :, :]
        lhs[:, :, T, :] = A[:, sl, :]
        rhs_hi = rhs.astype(bf16)
        rhs_lo = (rhs - rhs_hi.astype(np.float32)).astype(bf16)
        lhs_hi = lhs.astype(bf16)
        lhs_lo = (lhs - lhs_hi.astype(np.float32)).astype(bf16)
        ext_k = np.empty((B, SLOC, KP, N + HID), dtype=bf16)
        ext_k[:, :, 0 * K1:1 * K1, :N] = rhs_hi
        ext_k[:, :, 1 * K1:2 * K1, :N] = rhs_lo
        ext_k[:, :, 2 * K1:3 * K1, :N] = rhs_hi
        ext_k[:, :, 0 * K1:1 * K1, N:] = lhs_hi
        ext_k[:, :, 1 * K1:2 * K1, N:] = lhs_hi
        ext_k[:, :, 2 * K1:3 * K1, N:] = lhs_lo
        in_maps.append({
            "ext": np.ascontiguousarray(ext_k),
            "w2": mW2,
            "w2b": mW2b,
        })

    nc = _get_prog()
    trace = bool(int(os.environ.get("KERNEL_TRACE", "0")))
    try:
        res = run_bass_kernel_spmd(
            nc, in_maps, core_ids=list(range(NCORES)), trace=trace,
        )
    except ModuleNotFoundError:
        # axon NTFF profiling hook unavailable -> rerun without trace
        res = run_bass_kernel_spmd(
            nc, in_maps, core_ids=list(range(NCORES)), trace=False,
        )
    last_results = res

    msg_part = np.zeros((B, MSG, N), dtype=np.float32)
    for r in res.results:
        msg_part += r["msg_out"]

    msg_sum = np.transpose(msg_part, (0, 2, 1)) + N * mb2  # [B,N,MSG]
    node_feat = x[:, :, -1, :4]
    mi = np.concatenate([msg_sum, node_feat], axis=-1)     # [B,N,MSG+4]
    h2 = np.maximum(mi @ np.asarray(iW1, dtype=np.float32)
                    + np.asarray(ib1, dtype=np.float32), 0.0)
    out = h2 @ np.asarray(iW2, dtype=np.float32) + np.asarray(ib2, dtype=np.float32)
    return out.astype(np.float32)



# revision 63
# speedup vs baseline: 1.2019x; 1.2019x over previous
"""Trainium2 Bass kernel for the GNN message-passing model.

Math (reference):
    base[b,s,t,j] = x[b,s,t,j]            (j<4)
    extra[b,s,t,c] = x[b,s,t,4+c]
    h_pre[b,c,s,h] = sum_{t,j} base[b,s,t,j]*mW1[5t+j,h]
                   + sum_t extra[b,s,t,c]*mW1[5t+4,h] + mb1[h]
    msg_sum[b,c,:] = sum_s relu(h_pre[b,c,s,:]) @ mW2 + N*mb2
    out = MLP(concat(msg_sum, x[:,:,-1,:4]))

Decomposition:
  * A[b,s,h] = base-part + mb1 precomputed on host (tiny), folded into the
    per-pair matmul as extra contraction rows against a ones-row rhs.
  * Per (b,s) pair ONE fp8e4 DoubleRow matmul produces h_pre for all 512
    columns: 64 logical contraction rows as [32 partitions, 2 DR slots]:
      rows  0-10: (Whi,  xhi) + (Ahi, 1)
      rows 11-21: (Whi,  xlo) + (Alo, 1)
      rows 22-31: (Wlo,  xhi)
      rows 32-41: (Wlo,  xlo)          (exact product of fp8 hi/lo parts)
    DoubleRow runs at 0.5 cyc/row: ~92ns vs 213ns bf16. K is free on PE,
    so the 4-term split costs nothing extra.
  * The relu + sum-over-s of the 256 per-core [128,512] PSUM tiles is the
    real wall: the walrus BIR verifier only lets ACT and DVE touch PSUM
    (GPSIMD is rejected), so per b the 64 pairs are split
      - ~38 pairs: ACT relu (2-bank groups, bf16 out) + PE mW2-macc into a
        shared PSUM accumulator (PSUM accumulation = free s-sum).
      - ~26 pairs: DVE fused hacc = max(ps,0) + hacc (fp16 accumulator,
        so the end-of-b mW2 fold is a 2-byte matmul -- 4-byte matmuls
        only accept dst partition base 0 on this toolchain).
  * PSUM budget (8 banks): 2x2 double-buffered ACT groups, 3x1 DVE
    tiles, 1 shared macc bank. The per-b [32,512] mW2 accumulators
    stack in that one bank at partition bases {0,32,64,0} (b=3 reuses
    b=0's region after its output copy); start=True zeroing is
    per-partition so regions don't clobber each other.
  * Software pipelining: maccs trail their relu groups by 5 A-groups so
    the in-order PE never blocks on ACT; hacc folds and output copies
    are deferred into the next b; superchunk DMAs (16 pairs each) bound
    the serialized DGE/DMA-server time.
  * Sharding: data-parallel over source rows s (512 -> 64 per core); host
    sums the 8 partial msg tensors and runs the tiny update MLP in numpy.
"""

import os
import numpy as np

import concourse.bass as bass
import concourse.mybir as mybir
from concourse.tile import TileContext
from concourse.bass_utils import run_bass_kernel_spmd

B, N, T, F = 4, 512, 10, 516
HID, MSG = 128, 32
NCORES = 8
SLOC = N // NCORES          # source rows per core
CH = 4                      # pairs per 128-partition block (4 x 32)
F32 = mybir.dt.float32
BF16 = mybir.dt.bfloat16
FP8 = mybir.dt.float8e4
FP16 = mybir.dt.float16
DR = mybir.MatmulPerfMode.DoubleRow

# default per-b route mix over the 64 (b,s) pairs: ACT pairs (2-bank
# groups) vs DVE fused pairs; b=0 runs a DVE-heavier mix (36/28) to
# shorten the binding ACT stream
N_A, N_D = 38, 26

_prog = None
last_results = None

# Tile emits semaphore waits for same-engine WAW/RAW deps (e.g. an ACT op
# waiting on the ACT sem for a pool buffer recycled from an older ACT write).
# Compute engines execute strictly in order, so these waits are redundant --
# and they overflow the 1-slot sync-wait budget of several ISA structs
# (ACTIVATE, TensorScalarPtr). Strip them post-scheduling.
_STRIP_TYPES = {
    "InstActivation", "InstTensorScalarPtr", "InstTensorTensor",
    "InstTensorCopy", "InstTensorReduce", "InstMatmult", "InstMemSet",
}
_ENG2SEM = None


def _strip_self_waits(nc):
    global _ENG2SEM
    if _ENG2SEM is None:
        _ENG2SEM = {
            mybir.EngineType.PE: "PE_",
            mybir.EngineType.Activation: "Activation_",
            mybir.EngineType.DVE: "DVE_",
            mybir.EngineType.Pool: "Pool_",
        }
    for fn in nc.m.functions:
        for blk in fn.blocks:
            for inst in blk.instructions:
                if type(inst).__name__ not in _STRIP_TYPES:
                    continue
                si = inst.sync_info
                if si is None or not si.on_wait:
                    continue
                pre = _ENG2SEM.get(inst.engine)
                if pre is None:
                    continue
                kept = [w for w in si.on_wait if not (w.ant_name or "").startswith(pre)]
                if len(kept) != len(si.on_wait):
                    si.on_wait = kept
    # ACT/DVE ops recycling SBUF staging buffers carry a WAR wait on the
    # output DMA that read the buffer bufs generations ago (a full
    # b-iteration of slack); those DMAs are long complete, and the ISA
    # structs have a single sync-wait slot, so drop the DMA waits there.
    for fn in nc.m.functions:
        for blk in fn.blocks:
            for inst in blk.instructions:
                if type(inst).__name__ not in (
                        "InstActivation", "InstTensorScalarPtr",
                        "InstTensorCopy"):
                    continue
                si = inst.sync_info
                if si is None or not si.on_wait:
                    continue
                kept = [
                    w for w in si.on_wait
                    if not (w.ant_name or "").startswith(("DMAHW", "DMASW"))
                ]
                if len(kept) != len(si.on_wait):
                    si.on_wait = kept
    # Chunk-load DMAs: the WAR wait on the engine that read the recycled
    # buffer transitively dominates the WAW wait on the DMA that previously
    # filled it (that engine's reads each waited on that DMA themselves).
    eng_sems = ("PE_", "Activation_", "DVE_", "Pool_")
    for fn in nc.m.functions:
        for blk in fn.blocks:
            for inst in blk.instructions:
                if type(inst).__name__ != "InstDMACopy":
                    continue
                si = inst.sync_info
                if si is None or not si.on_wait:
                    continue
                has_eng = any((w.ant_name or "").startswith(eng_sems) for w in si.on_wait)
                if not has_eng:
                    continue
                kept = [
                    w for w in si.on_wait
                    if not (w.ant_name or "").startswith(("DMAHW", "DMASW"))
                ]
                if len(kept) != len(si.on_wait):
                    si.on_wait = kept
    # Kernel-tail Drain: waits on every DMA queue overflow the CTRL struct's
    # wait budget. Input-DMA waits are dominated by the engine waits (each
    # load was read by a compute engine before the drain); only the queues
    # carrying the output DMAs must be waited on directly.
    out_sems = set()
    for fn in nc.m.functions:
        for blk in fn.blocks:
            for inst in blk.instructions:
                if type(inst).__name__ != "InstDMACopy":
                    continue
                outs = getattr(inst, "outs", None) or []
                to_dram = any("msg_out" in (getattr(o, "memref", "") or "")
                              for o in outs)
                si = inst.sync_info
                if to_dram and si and si.on_update:
                    for u in si.on_update:
                        out_sems.add(u.ant_name)
    drain_split = 0
    for fn in nc.m.functions:
        for blk in fn.blocks:
            for ii in range(len(blk.instructions)):
                inst = blk.instructions[ii]
                if type(inst).__name__ != "InstDrain":
                    continue
                si = inst.sync_info
                if si is None or not si.on_wait or len(si.on_wait) <= 1:
                    continue
                waits = [
                    w for w in si.on_wait
                    if not (w.ant_name or "").startswith(("DMAHW", "DMASW"))
                    or w.ant_name in out_sems
                ]
                # split into a chain of drains with one wait each (the SP
                # CTRL struct has a single sync-wait slot)
                pre = []
                while len(waits) > 1:
                    chunk, waits = waits[:1], waits[1:]
                    d = mybir.InstDrain(
                        name=f"{inst.name}_split{drain_split}", ins=[], outs=[],
                        sync_info=mybir.SyncInfo(on_wait=chunk, on_update=[]),
                    )
                    d.engine = inst.engine
                    drain_split += 1
                    pre.append(d)
                si.on_wait = waits
                for d in reversed(pre):
                    blk.instructions.insert(ii, d)
                break


def _build_units(n_a=None, n_d=None):
    """Interleaved per-b schedule: list of ("A",[s,s2]) / ("D",[s]) / ("P",[s])."""
    if n_a is None:
        n_a, n_d = N_A, N_D
    counts = {"A": n_a // 2, "D": n_d, "P": 0}   # units (A unit = 2 pairs)
    total = sum(counts.values())
    cred = {k: 0.0 for k in counts}
    rem = dict(counts)
    kinds = []
    while len(kinds) < total:
        for k in counts:
            cred[k] += counts[k] / total
        pick = max((k for k in counts if rem[k] > 0), key=lambda k: cred[k])
        cred[pick] -= 1.0
        rem[pick] -= 1
        kinds.append(pick)
    # tail shaping: alternate D/P with A so no fused engine serializes at
    # the end, and finish on A units (their maccs overlap the epilogue)
    tail = kinds[-10:]
    non_a = [k for k in tail if k != "A"]
    a_s = [k for k in tail if k == "A"]
    shaped = []
    while non_a or a_s:
        if non_a:
            shaped.append(non_a.pop(0))
        if a_s:
            shaped.append(a_s.pop(0))
    kinds = kinds[:-10] + shaped
    if "D" in kinds and kinds[0] != "D":
        kinds.remove("D")
        kinds.insert(0, "D")
    units = []
    s = 0
    for k in kinds:
        if k == "A":
            units.append(("A", [s, s + 1]))
            s += 2
        else:
            units.append((k, [s]))
            s += 1
    assert s == SLOC
    return units


def _build_program():
    nc = bass.Bass(trn_type="TRN2")
    # superchunks: 16 pairs per DMA -> [B, 4, 128, 4, 2, 640]
    # partition = 32*q + row//2; free = (g_sub, slot, col)
    NSC = SLOC // 16
    ext = nc.dram_tensor("ext", [B, NSC, 128, 4, 2, 640], FP8,
                         kind="ExternalInput")
    w2b = nc.dram_tensor("w2b", [HID, MSG], BF16, kind="ExternalInput")
    w2r = nc.dram_tensor("w2r", [HID, MSG], FP16, kind="ExternalInput")
    msg_out = nc.dram_tensor("msg_out", [B * MSG, N], F32, kind="ExternalOutput")

    units_heavy_d = _build_units(36, 28)
    units_heavy_a = _build_units(38, 26)

    with TileContext(nc) as tc:
        with (
            tc.tile_pool(name="const", bufs=1) as constp,
            tc.tile_pool(name="big", bufs=3) as bigp,
            tc.tile_pool(name="scr", bufs=9) as scrp,
            tc.tile_pool(name="haccd", bufs=2) as hdp,
            tc.tile_pool(name="haccp", bufs=2) as hpp,
            tc.tile_pool(name="out", bufs=2) as outp,
            tc.tile_pool(name="pa", bufs=2, space="PSUM") as pa,
            tc.tile_pool(name="pd", bufs=3, space="PSUM") as pd,
            tc.tile_pool(name="pm", bufs=1, space="PSUM") as pm,
        ):
            macc = pm.tile([B * MSG, N], F32, tag="macc")
            RB = {0: 0, 1: MSG, 2: 2 * MSG, 3: 0}   # macc region per b
            w2bt = None
            w2rt = None
            pending_folds = []          # from the previous b
            pending = []                # delayed (global_gi, macc emitter)
            gga = [0]                   # global A-group counter

            def flush_pending(min_age=0, limit=99):
                done = 0
                while pending and done < limit and \
                        gga[0] - pending[0][0] >= min_age:
                    pending.pop(0)[1]()
                    done += 1

            for b in range(B):
                units = units_heavy_d if b < 1 else units_heavy_a
                def dma_chunk(g, b=b):
                    t = bigp.tile([128, 4, 2, 640], FP8, tag="big")
                    if b == 0 and g == 0:
                        # split the critical-path first load into 4 DMAs so
                        # the first pairs land with 1/4 the transfer latency
                        for gs in range(4):
                            nc.sync.dma_start(t[:, gs], ext[b, g, :, gs])
                    elif b == 0 and g in (1, 2, 3):
                        for gs in (0, 2):
                            nc.sync.dma_start(t[:, gs:gs + 2],
                                              ext[b, g, :, gs:gs + 2])
                    else:
                        nc.sync.dma_start(t[:], ext[b, g])
                    return t

                chunk_tiles = {}
                chunk_tiles[0] = dma_chunk(0)
                if NSC > 1:
                    chunk_tiles[1] = dma_chunk(1)
                if b == 0:
                    # weights load after the first two chunks: they are not
                    # needed until the first macc, ~8 units in
                    chunk_tiles[2] = dma_chunk(2)
                    w2bt = constp.tile([HID, MSG], BF16, tag="w2bt")
                    nc.sync.dma_start(w2bt[:], w2b[:])
                    w2rt = constp.tile([HID, MSG], FP16, tag="w2rt")
                    nc.sync.dma_start(w2rt[:], w2r[:])

                def pair_aps(s):
                    g, r = s // 16, s % 16
                    gs, q = r // CH, r % CH
                    t = chunk_tiles[g]
                    lhsT = t[32 * q:32 * q + 32, gs, :, 512:512 + HID]
                    rhs = t[32 * q:32 * q + 32, gs, :, 0:512]
                    return lhsT, rhs, (32 * q, 0)

                n_ga_b = sum(1 for k, _ in units if k == "A")
                nmm = 0                 # macc count for this b
                n_emitted_a = 0         # A groups emitted so far
                hacc_d = None
                next_prefetch = 3 if b == 0 else 2
                rem_d = N_D
                late_folds = []         # last-b folds, delayed a few units
                last_b = b == B - 1

                for ui, (kind, ss) in enumerate(units):
                    # prefetch the superchunk for pairs ~20 ahead
                    last_s = ss[-1]
                    while next_prefetch <= min((last_s + 20) // 16, NSC - 1):
                        chunk_tiles[next_prefetch] = dma_chunk(next_prefetch)
                        next_prefetch += 1
                    # previous b's hacc fold, once this b is warmed up;
                    # any still-pending maccs from that b flush first
                    if ui in (8, 14) and pending_folds:
                        flush_pending(min_age=n_emitted_a)
                        pending_folds.pop(0)()
                    while late_folds and late_folds[0][0] <= ui:
                        late_folds.pop(0)[1]()

                    if kind == "A":
                        ps2 = pa.tile([HID, 2, 512], F32, tag="ps2")
                        for j, s in enumerate(ss):
                            lhsT, rhs, tp = pair_aps(s)
                            nc.tensor.matmul(ps2[:, j, :], lhsT, rhs,
                                             start=True, stop=True,
                                             perf_mode=DR, tile_position=tp)
                        scr = scrp.tile([HID, 2, 512], BF16, tag="scr")
                        nc.scalar.activation(
                            scr[:], ps2[:], mybir.ActivationFunctionType.Relu)

                        gi = n_emitted_a
                        ggi = gga[0]

                        def emit_macc(scr, j, b=b, gi=gi):
                            def f():
                                nonlocal nmm
                                is_last = (last_b and gi == n_ga_b - 1
                                           and j == 1)
                                nc.tensor.matmul(
                                    macc[RB[b]:RB[b] + MSG, :],
                                    w2bt[:], scr[:, j, :],
                                    start=(nmm == 0), stop=is_last,
                                    skip_group_check=True,
                                    tile_position=(0, RB[b]))
                                nmm += 1
                            return (gi, f)
                        pending.append((ggi, emit_macc(scr, 0)[1]))
                        pending.append((ggi, emit_macc(scr, 1)[1]))
                        n_emitted_a += 1
                        gga[0] += 1
                    elif kind == "D":
                        ps = pd.tile([HID, 512], F32, tag="psd")
                        lhsT, rhs, tp = pair_aps(ss[0])
                        nc.tensor.matmul(ps[:], lhsT, rhs, start=True,
                                         stop=True, perf_mode=DR,
                                         tile_position=tp)
                        if hacc_d is None:
                            hacc_d = hdp.tile([HID, 512], FP16, tag="haccd")
                            nc.vector.tensor_scalar(
                                hacc_d[:], ps[:], 0.0, None,
                                op0=mybir.AluOpType.max)
                        else:
                            nc.vector.scalar_tensor_tensor(
                                hacc_d[:], ps[:], 0.0, hacc_d[:],
                                op0=mybir.AluOpType.max,
                                op1=mybir.AluOpType.add)
                        rem_d -= 1
                        if last_b and rem_d == 0:
                            def lf(b=b, hacc_d=hacc_d):
                                nc.tensor.matmul(
                                    macc[RB[b]:RB[b] + MSG, :], w2rt[:],
                                    hacc_d[:],
                                    start=False, stop=False,
                                    skip_group_check=True,
                                    tile_position=(0, RB[b]))
                            late_folds.append((ui + 3, lf))
                    # trickle out delayed maccs AFTER the unit's route
                    # matmuls so the drain engines always get fed first
                    if ui >= len(units) - 6:
                        flush_pending(min_age=1, limit=3)
                    else:
                        flush_pending(min_age=5, limit=2)
                if last_b:
                    for _, fn in late_folds:
                        fn()
                    late_folds.clear()
                    flush_pending()

                if not last_b:
                    # fold the DVE accumulator through mW2 (f32r PE),
                    # delayed into the next b so its engines never idle
                    def fold_d(b=b, hacc_d=hacc_d):
                        nc.tensor.matmul(
                            macc[RB[b]:RB[b] + MSG, :], w2rt[:],
                            hacc_d[:],
                            start=False, stop=True, skip_group_check=True,
                            tile_position=(0, RB[b]))

                    def copy_out(b=b):
                        if b == 1:
                            # b0+b1 output while b=2 computes
                            ot = outp.tile([2 * MSG, N], F32, tag="ot")
                            nc.vector.tensor_copy(ot[:], macc[0:2 * MSG, :])
                            nc.sync.dma_start(msg_out[0:2 * MSG, :], ot[:])
                        elif b == 2:
                            # b2 output while b=3 computes
                            ot = outp.tile([MSG, N], F32, tag="ot2")
                            nc.vector.tensor_copy(ot[:], macc[2 * MSG:3 * MSG, :])
                            nc.sync.dma_start(msg_out[2 * MSG:3 * MSG, :],
                                              ot[:])
                    pending_folds.extend([fold_d, copy_out])

            for fn in pending_folds:
                fn()
            ot = outp.tile([MSG, N], F32, tag="ot3")
            nc.vector.tensor_copy(ot[:], macc[0:MSG, :])
            nc.sync.dma_start(msg_out[3 * MSG:, :], ot[:])
    _strip_self_waits(nc)
    return nc


def _get_prog():
    global _prog
    if _prog is None:
        _prog = _build_program()
    return _prog


def kernel(x, mW1, mb1, mW2, mb2, iW1, ib1, iW2, ib2):
    global last_results
    import ml_dtypes
    fp8 = ml_dtypes.float8_e4m3fn
    bf16 = ml_dtypes.bfloat16

    x = np.ascontiguousarray(np.asarray(x, dtype=np.float32))
    mW1 = np.asarray(mW1, dtype=np.float32)
    mb1 = np.asarray(mb1, dtype=np.float32)
    mW2 = np.ascontiguousarray(np.asarray(mW2, dtype=np.float32))
    mb2 = np.asarray(mb2, dtype=np.float32)

    # host prep: A[b,s,h] = base_flat @ W1b + mb1 (tiny)
    base = x[:, :, :, :4]                                  # [B,N,T,4]
    base_flat = base.reshape(B, N, T * 4)
    W1b = mW1.reshape(T, 5, HID)[:, :4, :].reshape(T * 4, HID)
    W1x = np.ascontiguousarray(mW1.reshape(T, 5, HID)[:, 4, :])   # [T,HID]
    A = (base_flat @ W1b + mb1).astype(np.float32)         # [B,N,HID]

    def q8(a):
        return a.astype(fp8).astype(np.float32)

    Whi = q8(W1x); Wlo = (W1x - Whi)                       # [T,HID]
    Ahi = q8(A); Alo = (A - Ahi)                           # [B,N,HID]
    E = x[:, :, :, 4:4 + N]                                # [B,N,T,N]
    Ehi = q8(E); Elo = (E - Ehi)

    in_maps = []
    for k in range(NCORES):
        sl = slice(k * SLOC, (k + 1) * SLOC)
        R64 = np.zeros((B, SLOC, 64, 640), dtype=fp8)
        # rhs rows (cols 0:512)
        R64[:, :, 0:10, :512] = Ehi[:, sl].transpose(0, 1, 2, 3)
        R64[:, :, 10, :512] = 1.0
        R64[:, :, 11:21, :512] = Elo[:, sl]
        R64[:, :, 21, :512] = 1.0
        R64[:, :, 22:32, :512] = Ehi[:, sl]
        R64[:, :, 32:42, :512] = Elo[:, sl]
        # lhsT rows (cols 512:640)
        R64[:, :, 0:10, 512:] = Whi[None, None]
        R64[:, :, 10, 512:] = Ahi[:, sl]
        R64[:, :, 11:21, 512:] = Whi[None, None]
        R64[:, :, 21, 512:] = Alo[:, sl]
        R64[:, :, 22:32, 512:] = Wlo[None, None]
        R64[:, :, 32:42, 512:] = Wlo[None, None]
        # [B, SLOC, 64rows, 640] -> [B, 4, 128, 4, 2, 640] superchunks:
        #   s = 16*G + 4*gs + q; row r -> (partition 32*q + r//2, slot r%2)
        ext_k = (R64.reshape(B, 4, 4, 4, 32, 2, 640)
                 .transpose(0, 1, 3, 4, 2, 5, 6)
                 .reshape(B, 4, 128, 4, 2, 640))
        in_maps.append({
            "ext": np.ascontiguousarray(ext_k),
            "w2b": mW2.astype(bf16),
            "w2r": mW2.astype(np.float16),
        })

    nc = _get_prog()
    trace = bool(int(os.environ.get("KERNEL_TRACE", "0")))
    try:
        res = run_bass_kernel_spmd(
            nc, in_maps, core_ids=list(range(NCORES)), trace=trace,
        )
    except ModuleNotFoundError:
        res = run_bass_kernel_spmd(
            nc, in_maps, core_ids=list(range(NCORES)), trace=False,
        )
    last_results = res

    msg_part = np.zeros((B * MSG, N), dtype=np.float32)
    for r in res.results:
        msg_part += r["msg_out"]
    msg_part = msg_part.reshape(B, MSG, N)

    msg_sum = np.transpose(msg_part, (0, 2, 1)) + N * mb2  # [B,N,MSG]
    node_feat = x[:, :, -1, :4]
    mi = np.concatenate([msg_sum, node_feat], axis=-1)     # [B,N,MSG+4]
    h2 = np.maximum(mi @ np.asarray(iW1, dtype=np.float32)
                    + np.asarray(ib1, dtype=np.float32), 0.0)
    out = h2 @ np.asarray(iW2, dtype=np.float32) + np.asarray(ib2, dtype=np.float32)
    return out.astype(np.float32)
